# revision 15
# baseline (speedup 1.0000x reference)
"""Trainium2 Bass kernel for nn_CPCircuitLayer.

Math: with all_indices the full cartesian grid (s = n // H, h = n % H),
    out[b, s, h] = sum_r seq_emb[b,s,r] * hid_emb[b,h,r] * cp[r]
                 = (seq_emb[b] @ diag(cp) @ hid_emb[b].T)[s, h]
where seq_emb[b] = X_b @ seq_W.T  (X_b = hidden_states[b], contract H)
      hid_emb[b] = X_b.T @ hid_W.T                        (contract S)

Sharding: 8 cores = (batch b, seq half) pairs. Each core receives X_b
fully (the hid factor contracts over all of S) with rows rotated so its
own seq half comes first, plus a host-transposed copy of that half
(xt = X_b[half].T), and computes
    hid_embT = (hid_W*cp) @ X_b          [R, H]
    seq_embT = seq_W @ X_b[half].T       [R, S/2]
    out_half = seq_embT.T @ hid_embT     [S/2, H]
writing its [512, 1024] slice of the output.

Everything on device is float16 (halves DMA traffic vs fp32r; PE
streams 1 row/cycle when HBM DMA is quiet, 1 row/2 cycles while input
DMA writes SBUF). Empirical DMA model from traces: the two HWDGE
queues drain ~55 descriptors/us each when both are active (~110 solo),
one descriptor per SBUF partition row, so byte rate is descriptor-size
bound until the ~435 GB/s DDR cap; hence every transfer here uses
>=4KB contiguous rows. The tiny weight tensor rides in the same DMA as
the first two x k-tiles (5KB rows) instead of paying its own 128
descriptors. The fp16 output is written as two 512KB chunks with 4KB
rows and upcast to f32 on the host. Total rounding error ~1e-3
relative, far under the 2e-2 gate.
"""

import numpy as np

B, S, H, R = 4, 1024, 1024, 32
N_CORES = 8
SH = S // 2    # seq rows per core
KT = S // 128  # k-tiles over the contraction dims
MT = SH // 128  # row tiles in this core's seq half
WC = 2 * KT * R  # 512 w columns (sw | hw)

_compiled = {}


def _np_fallback(hidden_states, all_indices, seq_W, hid_W, cp_weight):
    seq_emb = np.einsum("bsh,rh->bsr", hidden_states, seq_W)
    hid_emb = np.einsum("bsh,rs->bhr", hidden_states, hid_W)
    s_idx = all_indices[:, 0].astype(np.int64)
    h_idx = all_indices[:, 1].astype(np.int64)
    g_seq = seq_emb[:, s_idx, :]
    g_hid = hid_emb[:, h_idx, :]
    out = np.einsum("bnr,bnr,r->bn", g_seq, g_hid, cp_weight[0])
    return out.reshape(B, S, H).astype(np.float32)


def _tile128(a):
    """[K*128, N] -> [128, K*N] with k-tiles adjacent in the free dim."""
    k = a.shape[0] // 128
    return np.ascontiguousarray(
        a.reshape(k, 128, a.shape[1]).transpose(1, 0, 2).reshape(128, -1))


def _wtile(w):
    """[K, R] -> [128, KT*R] tile layout, partition-contiguous."""
    return np.ascontiguousarray(
        w.reshape(KT, 128, R).transpose(1, 0, 2).reshape(128, KT * R))


def build_raw_program():
    import contextlib

    import concourse.bass as bass
    import concourse.mybir as mybir

    f32 = mybir.dt.float32
    f16 = mybir.dt.float16

    nc = bass.Bass("TRN2", target_bir_lowering=False, debug=False,
                   num_devices=N_CORES, enable_partition_id=False)

    # x carries [w (512 cols) | x k-tiles (KT*1024 cols)] fused
    x_d = nc.dram_tensor("x", [128, WC + KT * H], f16, kind="ExternalInput")
    xt_d = nc.dram_tensor("xt", [128, KT * SH], f16, kind="ExternalInput")
    out_d = nc.dram_tensor("out", [128, MT * H], f16, kind="ExternalOutput")

    with contextlib.ExitStack() as _xs:
        E = _xs.enter_context
        wx_t = E(nc.sbuf_tensor([128, WC + KT * H], f16))
        xt_t = E(nc.sbuf_tensor([128, KT, SH], f16))
        hid_sb = E(nc.sbuf_tensor([R, H], f16))
        seq_sb = E(nc.sbuf_tensor([R, SH], f16))
        o_sb = E(nc.sbuf_tensor([128, MT, H], f16))
        hid_ps = E(nc.psum_tensor([R, H], f32))        # 2 banks
        seq_ps = E(nc.psum_tensor([R, SH], f32))       # 1 bank
        o_ps = [E(nc.psum_tensor(f"o_ps{i}", [128, 512], f32))
                for i in range(5)]                     # 5 banks
        dma_sem = E(nc.semaphore("dma_sem"))
        pe_sem = E(nc.semaphore("pe_sem"))
        dve_sem = E(nc.semaphore("dve_sem"))
        act_sem = E(nc.semaphore("act_sem"))
        x_sem = [E(nc.semaphore(f"x_sem{j}")) for j in range(5)]
        xt_sem = [E(nc.semaphore(f"xt_sem{j}")) for j in range(2)]
        block = E(nc.Block(no_gpsimd_drain=True))

        sw = lambda k: wx_t.ap()[:, k * R:(k + 1) * R]
        hw = lambda k: wx_t.ap()[:, KT * R + k * R:KT * R + (k + 1) * R]
        xk = lambda k: wx_t.ap()[:, WC + k * H:WC + (k + 1) * H]

        # DMA plan (seq data early so its copies overlap; k7 last, small):
        #  Q_sync: w+x_k0 | xt_k4..7 | x_k3k4 | x_k7
        #  Q_act : xt_k0..3 | x_k1k2 | x_k5k6
        # PE order (pe_sem counts; hid k = 2 matmuls n0,n1; seq k = 1):
        #  [wx0] hid k0      -> 1..2
        #  [xt0] seq k0..3   -> 3..6
        #  [xt1] seq k4..7   -> 7..10   (seq done @10)
        #  [x1]  hid k1,k2   -> 11..14
        #  [x2]  hid k3,k4   -> 15..18
        #  [x3]  hid k5,k6   -> 19..22
        #  [x4]  hid k7      -> 23..24  (hid n0 done @23, n1 @24)
        #  out j=0..7        -> 25..32
        SEQ_DONE = 10
        HID_N0_DONE = 23
        HID_N1_DONE = 24

        @block.sync
        def _(sync):
            sync.dma_start(out=wx_t.ap()[:, 0:WC + H],
                           in_=x_d[:, 0:WC + H]).then_inc(x_sem[0], 16)
            sync.dma_start(out=xt_t.ap()[:, 4:8, :],
                           in_=xt_d[:, 4 * SH:8 * SH]).then_inc(xt_sem[1], 16)
            sync.dma_start(out=wx_t.ap()[:, WC + 3 * H:WC + 5 * H],
                           in_=x_d[:, WC + 3 * H:WC + 5 * H]
                           ).then_inc(x_sem[2], 16)
            sync.dma_start(out=wx_t.ap()[:, WC + 7 * H:WC + 8 * H],
                           in_=x_d[:, WC + 7 * H:WC + 8 * H]
                           ).then_inc(x_sem[4], 16)
            # out chunk c covers m-pair (2c, 2c+1): needs copies j=4c..4c+3
            sync.wait_ge(dve_sem, 7)
            sync.wait_ge(act_sem, 3)
            sync.dma_start(out=out_d[:, 0:2 * H],
                           in_=o_sb.ap()[:, 0:2, :]).then_inc(dma_sem, 16)
            sync.wait_ge(dve_sem, 9)
            sync.wait_ge(act_sem, 5)
            sync.dma_start(out=out_d[:, 2 * H:4 * H],
                           in_=o_sb.ap()[:, 2:4, :]).then_inc(dma_sem, 16)
            sync.wait_ge(dma_sem, 32)

        @block.scalar
        def _(scalar):
            scalar.dma_start(out=xt_t.ap()[:, 0:4, :],
                             in_=xt_d[:, 0:4 * SH]).then_inc(xt_sem[0], 16)
            scalar.dma_start(out=wx_t.ap()[:, WC + H:WC + 3 * H],
                             in_=x_d[:, WC + H:WC + 3 * H]
                             ).then_inc(x_sem[1], 16)
            scalar.dma_start(out=wx_t.ap()[:, WC + 5 * H:WC + 7 * H],
                             in_=x_d[:, WC + 5 * H:WC + 7 * H]
                             ).then_inc(x_sem[3], 16)
            # dummy copy (garbage data) pulls the lazy ACT table load early
            nc.scalar.copy(o_sb.ap()[:, 0, 0:R], o_sb.ap()[:, 1, 0:R])
            scalar.wait_ge(pe_sem, HID_N1_DONE)
            nc.scalar.copy(
                hid_sb.ap()[:, 512:1024],
                hid_ps.ap()[:, 512:1024]).then_inc(act_sem, 1)
            for j in range(1, 2 * MT, 2):   # odd out copies -> act 2..5
                m, n = divmod(j, 2)
                scalar.wait_ge(pe_sem, 24 + j + 1)
                nc.scalar.copy(
                    o_sb.ap()[:, m, n * 512:(n + 1) * 512],
                    o_ps[j % 5].ap(),
                ).then_inc(act_sem, 1)

        @block.tensor
        def _(tensor):
            def hid_k(k, start=False, stop=False):
                for n in range(2):
                    nc.tensor.matmul(
                        hid_ps.ap()[:, n * 512:(n + 1) * 512],
                        hw(k), xk(k)[:, n * 512:(n + 1) * 512],
                        start=start, stop=stop,
                    ).then_inc(pe_sem, 1)

            def seq_k(k, start=False, stop=False):
                nc.tensor.matmul(
                    seq_ps.ap(), sw(k), xt_t.ap()[:, k, :],
                    start=start, stop=stop,
                ).then_inc(pe_sem, 1)

            tensor.wait_ge(x_sem[0], 16)
            hid_k(0, start=True)                       # pe 1..2
            # seq k4..7 first: xt_k4..7 rides the early (sync) queue
            tensor.wait_ge(xt_sem[1], 16)
            seq_k(4, start=True)
            for k in range(5, 8):
                seq_k(k)                               # pe 3..6
            tensor.wait_ge(xt_sem[0], 16)
            for k in range(3):
                seq_k(k)
            seq_k(3, stop=True)                        # pe 7..10
            tensor.wait_ge(x_sem[1], 16)
            hid_k(1)
            hid_k(2)                                   # pe 11..14
            tensor.wait_ge(x_sem[2], 16)
            hid_k(3)
            hid_k(4)                                   # pe 15..18
            tensor.wait_ge(x_sem[3], 16)
            hid_k(5)
            hid_k(6)                                   # pe 19..22
            tensor.wait_ge(x_sem[4], 16)
            hid_k(7, stop=True)                        # pe 23..24

            # final stage: out[j] = seq_sb[:, m-chunk].T @ hid_sb[:, n-chunk]
            for j in range(2 * MT):
                m, n = divmod(j, 2)
                if j == 0:
                    tensor.wait_ge(dve_sem, 5)   # all seq copies + hid n0
                if j == 1:
                    tensor.wait_ge(act_sem, 1)   # hid n1 copy
                if j == 5:
                    tensor.wait_ge(dve_sem, 6)   # WAR: bank of j=0 copied
                if j == 6:
                    tensor.wait_ge(act_sem, 2)   # WAR: bank of j=1 copied
                if j == 7:
                    tensor.wait_ge(dve_sem, 7)   # WAR: bank of j=2 copied
                nc.tensor.matmul(
                    o_ps[j % 5].ap(),
                    seq_sb.ap()[:, m * 128:(m + 1) * 128],
                    hid_sb.ap()[:, n * 512:(n + 1) * 512],
                    start=True, stop=True,
                ).then_inc(pe_sem, 1)

        @block.vector
        def _(vector):
            vector.wait_ge(pe_sem, SEQ_DONE)
            for m in range(MT):   # chunked seq copy -> dve 1..4
                nc.vector.tensor_copy(
                    seq_sb.ap()[:, m * 128:(m + 1) * 128],
                    seq_ps.ap()[:, m * 128:(m + 1) * 128],
                ).then_inc(dve_sem, 1)
            vector.wait_ge(pe_sem, HID_N0_DONE)
            nc.vector.tensor_copy(
                hid_sb.ap()[:, 0:512],
                hid_ps.ap()[:, 0:512]).then_inc(dve_sem, 1)   # dve 5
            for j in range(0, 2 * MT, 2):   # even out copies -> dve 6..9
                m, n = divmod(j, 2)
                vector.wait_ge(pe_sem, 24 + j + 1)
                nc.vector.tensor_copy(
                    o_sb.ap()[:, m, n * 512:(n + 1) * 512],
                    o_ps[j % 5].ap(),
                ).then_inc(dve_sem, 1)

    return nc


def _get_program():
    if "nc" not in _compiled:
        _compiled["nc"] = build_raw_program()
    return _compiled["nc"]


def _make_in_maps(hidden_states, seq_W, hid_W, cp_weight):
    swT = _wtile(np.ascontiguousarray(seq_W.T, dtype=np.float16))  # [128, 256]
    hwT_rows = np.ascontiguousarray(
        (hid_W * cp_weight[0][:, None]).T, dtype=np.float16)       # [S, R]
    # per-half row rotation: own seq half first (hid contraction over S is
    # order-invariant as long as x rows and hw rows permute together)
    w_rot = [
        np.concatenate([swT, _wtile(np.concatenate(
            [hwT_rows[half * SH:], hwT_rows[:half * SH]], axis=0))], axis=1)
        for half in range(2)
    ]
    in_maps = []
    for c in range(N_CORES):
        b, half = divmod(c, 2)
        xb = hidden_states[b].astype(np.float16)
        if half:
            xb = np.concatenate([xb[SH:], xb[:SH]], axis=0)
        in_maps.append({
            "x": np.ascontiguousarray(
                np.concatenate([w_rot[half], _tile128(xb)], axis=1)),
            "xt": _tile128(np.ascontiguousarray(xb[:SH, :].T)),
        })
    return in_maps


def kernel(hidden_states, all_indices, seq_W, hid_W, cp_weight):
    hidden_states = np.asarray(hidden_states, dtype=np.float32)
    seq_W = np.asarray(seq_W, dtype=np.float32)
    hid_W = np.asarray(hid_W, dtype=np.float32)
    cp_weight = np.asarray(cp_weight, dtype=np.float32)
    idx = np.asarray(all_indices)

    # The reference's all_indices is always the full cartesian grid; verify
    # cheaply and fall back to a host path if ever not.
    n = np.arange(S * H, dtype=idx.dtype)
    if idx.shape != (S * H, 2) or not (
        np.array_equal(idx[:, 0], n // H) and np.array_equal(idx[:, 1], n % H)
    ):
        return _np_fallback(hidden_states, idx, seq_W, hid_W, cp_weight)

    from concourse.bass_utils import run_bass_kernel_spmd

    nc = _get_program()
    in_maps = _make_in_maps(hidden_states, seq_W, hid_W, cp_weight)
    res = run_bass_kernel_spmd(nc, in_maps, list(range(N_CORES)))

    out = np.empty((B, S, H), dtype=np.float32)
    for c in range(N_CORES):
        b, half = divmod(c, 2)
        ot = res.results[c]["out"]  # [128, MT*H] f16, tiled by m
        out[b, half * SH:(half + 1) * SH, :] = (
            ot.reshape(128, MT, H).transpose(1, 0, 2)
            .reshape(SH, H).astype(np.float32))
    return out
